# revision 2
# baseline (speedup 1.0000x reference)
"""CoOccurrenceLayer Trainium2 kernel v4 (8 NeuronCores, data-parallel).

out[p] = sum_dq filt[dq] * co[idx[p], idx[p+dq]] * x[p+dq],
idx = 16-bin quantization of exp(x) normalized by global min/max.

Structure:
  * 5-op binning, exact: ACT Exp -> ACT Relu(e*s1 + s2) in [0,16)
    (floor(relu(t-eps)) == floor(|t-eps|)) -> gpsimd +16 -> DVE
    mantissa-AND floor on [16,32) -> gpsimd f16 copy. Run on the V-grid
    x (scatter) and an out-grid packed copy (select mask).
  * ch-major conv: per image, 4 column-quarters; per-bin 5x5 convs as
    PSUM-accumulated matmuls, ACT-evacuated into a j-interleaved slab;
    strip phase of quarter q overlaps conv of quarter q+1.
  * Input-adaptive bin skipping: host groups the 64 images into 8
    slots so rare bins cluster; absent bins skip scatter+conv and are
    zeroed in the per-slot mix weights.
  * Strip phase per half-quarter: sync-issued DMA-xbar transposes
    (ACT-issued transposes return stale data); mask replication via
    DVE stride-0 broadcast; mix on PE; fused compare-and-multiply
    select reading PSUM directly; 16->1 reduce on PE; output DMAs
    dispatched from ACT.
"""

import sys

sys.path.insert(0, "/opt/trn_rl_repo")

import numpy as np

import concourse.bacc as bacc
import concourse.mybir as mybir
import concourse.tile as tile
from concourse import bass_utils
from concourse.ap import AP

# ---------------------------------------------------------------- constants
B, HH, WW = 64, 512, 512
NCORES = 8
BPC = B // NCORES
NQ = 16
EPS = 1e-5

AV, BV = 65, 33            # V-grid blocks of 8x16 (padded input 520x528)
NV = AV * BV               # 2145
AO, BO = 64, 32            # out grid
NO = AO * BO               # 2048 out-blocks
NG = NO // 8               # 256 block-groups of 8

F16 = mybir.dt.float16
F32 = mybir.dt.float32
U32 = mybir.dt.uint32
ALU = mybir.AluOpType
AFT = mybir.ActivationFunctionType

FLOOR_MASK = 0xFFF80000    # clears 19 low mantissa bits: exact floor on [16,32)
GP_SCATTER = 3             # bins scattered on gpsimd (2-op form)


# ------------------------------------------------------- host-side binning
def host_idx(x, m, M):
    e = np.exp(x.astype(np.float32)).astype(np.float32)
    norm = ((e - m) / M).astype(np.float32)
    t = (norm * np.float32(16.0)).astype(np.float32)
    return np.floor(np.abs(t - np.float32(EPS))).astype(np.int32), t


def group_images(x, m, M):
    """Group 64 images into 8 slots minimizing per-slot bin unions."""
    idx, t = host_idx(x[:, 0], m, M)
    y = t - np.float32(EPS)
    presence = []
    for i in range(B):
        ids = set(np.unique(idx[i]).tolist())
        yi = y[i]
        near = np.abs(yi - np.round(yi)) < 1e-3
        if near.any():
            for v in np.unique(np.round(yi[near]).astype(np.int32)):
                if 1 <= v <= 15:
                    ids.add(int(v))
                    ids.add(int(v - 1))
        presence.append(frozenset(ids))

    order = sorted(range(B), key=lambda i: -len(presence[i]))
    groups = [[] for _ in range(BPC)]
    unions = [set() for _ in range(BPC)]
    for i in order:
        best, bcost = None, None
        for g in range(BPC):
            if len(groups[g]) >= NCORES:
                continue
            cost = (len(unions[g] | presence[i]) - len(unions[g]),
                    len(groups[g]))
            if bcost is None or cost < bcost:
                best, bcost = g, cost
        groups[best].append(i)
        unions[best] |= presence[i]
    slot_bins = [sorted(u) for u in unions]
    return groups, slot_bins


# ------------------------------------------------------- static PE weights
def build_weights(co, filt, slot_bins):
    """wt [128, (5+BPC)*128] f16: 4 conv lhsT, 1 reduce lhsT, BPC mix lhsT."""
    nw = 5 + BPC
    W = np.zeros((nw, 128, 128), np.float32)
    for da in range(2):
        for db in range(2):
            v = da * 2 + db
            for kr in range(8):
                for kc in range(16):
                    for mr in range(8):
                        for mc in range(16):
                            dr = 8 * da - 2 + kr - mr
                            dc = 16 * db - 2 + kc - mc
                            if -2 <= dr <= 2 and -2 <= dc <= 2:
                                W[v, kr * 16 + kc, mr * 16 + mc] = filt[dr + 2, dc + 2]
    for i in range(NQ):
        for blk8 in range(8):
            W[4, i * 8 + blk8, blk8] = 1.0
    for s in range(BPC):
        bins = set(slot_bins[s])
        for j in range(NQ):
            if j not in bins:
                continue
            for blk8 in range(8):
                for i in range(NQ):
                    W[5 + s, j * 8 + blk8, i * 8 + blk8] = co[i, j]
    return np.ascontiguousarray(
        W.astype(np.float16).transpose(1, 0, 2).reshape(128, nw * 128)
    )


# ------------------------------------------------------- device program
def _binchain(nc, p_tmp, src_f32, out_f16, s2s_ap, m16_ap, s1, n, tag):
    """K = floor(|t - eps|) from x (f32), exact vs the reference chain.

    y' = relu(e*s1 + s2 + 16) lies in [16,32) except the global-min pixel
    (y' in [16-eps,16) -> AND gives 15.5); the final Relu(K'-16) copy maps
    that case to bin 0, which matches floor(|t-eps|) exactly."""
    nc.scalar.activation(src_f32[:], src_f32[:], AFT.Exp)
    nc.scalar.activation(src_f32[:], src_f32[:], AFT.Relu, bias=s2s_ap,
                         scale=float(s1))
    nc.vector.tensor_scalar(src_f32[:], src_f32[:], 16.0, None, ALU.add)
    nc.vector.tensor_scalar(
        src_f32[:].bitcast(U32), src_f32[:].bitcast(U32), FLOOR_MASK, None,
        ALU.bitwise_and,
    )
    nc.scalar.activation(out_f16[:], src_f32[:], AFT.Relu, bias=m16_ap)


def _strip_half(nc, img, g0, wt, ko, cs, h, piv_ap, pools, o_d):
    p_ct, p_rp, p_it, p_pr, p_o16, p_mps, p_ops = pools
    ct = p_ct.tile([128, 32, 128], F16, tag="ct")
    nc.sync.dma_start_transpose(ct[:], cs[:, h * 4096 : (h + 1) * 4096])
    rp = p_rp.tile([128, 4096], F16, tag="rp")
    kv = ko[:]
    rsrc = AP(kv.tensor, kv.offset + g0 * 8,
              [kv.ap[0], [8, 32], [0, 16], [1, 8]])
    rv = rp[:]
    rdst = AP(rv.tensor, rv.offset,
              [rv.ap[0], [128, 32], [8, 16], [1, 8]])
    nc.scalar.copy(rdst, rsrc)
    it = p_it.tile([128, 32, 128], F16, tag="it")
    nc.sync.dma_start_transpose(it[:], rp[:])
    mk = p_rp.tile([128, 4096], F16, tag="mk")
    nc.vector.tensor_scalar(
        mk[:], it[:].rearrange("p a b -> p (a b)"), piv_ap, None, ALU.is_equal,
    )
    o16 = p_o16.tile([8, 4096], F16, tag="o16")
    ctf = ct[:].rearrange("p a b -> p (a b)")
    for half in range(4):
        mp = p_mps.tile([128, 1024], F32, tag="mp")
        for q in range(2):
            c0 = half * 1024 + q * 512
            nc.tensor.matmul(
                mp[:, q * 512 : (q + 1) * 512],
                wt[:, (5 + img) * 128 : (6 + img) * 128],
                ctf[:, c0 : c0 + 512],
                start=True, stop=True,
            )
        pr = p_pr.tile([128, 1024], F16, tag="pr")
        nc.vector.tensor_tensor(
            pr[:], mk[:, half * 1024 : (half + 1) * 1024], mp[:], ALU.mult,
        )
        for q in range(2):
            ob = p_ops.tile([8, 512], F32, tag="ob")
            nc.tensor.matmul(
                ob[:], wt[:, 4 * 128 : 4 * 128 + 8],
                pr[:, q * 512 : (q + 1) * 512],
                start=True, stop=True,
            )
            nc.scalar.copy(
                o16[:, half * 1024 + q * 512 : half * 1024 + (q + 1) * 512],
                ob[:],
            )
    nc.scalar.dma_start(o_d[img, :, g0 * 128 : g0 * 128 + 4096], o16[:])


def build_program(s1, s2, slot_bins):
    nc = bacc.Bacc("TRN2", target_bir_lowering=False, debug=False)
    x_d = nc.dram_tensor("x", [BPC, 128, NV], F32, kind="ExternalInput").ap()
    x16_d = nc.dram_tensor("x16", [BPC, 128, NV], F16, kind="ExternalInput").ap()
    xo_d = nc.dram_tensor("xo", [BPC, 128, NO], F32, kind="ExternalInput").ap()
    w_d = nc.dram_tensor("wt", [128, (5 + BPC) * 128], F16, kind="ExternalInput").ap()
    av_d = nc.dram_tensor("av", [128, 3], F32, kind="ExternalInput").ap()
    o_d = nc.dram_tensor("out", [BPC, 8, NG * 128], F16, kind="ExternalOutput").ap()

    with tile.TileContext(nc) as tc:
        with (
            tc.tile_pool(name="wp", bufs=1) as p_w,
            tc.tile_pool(name="xs", bufs=1) as p_xs,
            tc.tile_pool(name="xop", bufs=1) as p_xo,
            tc.tile_pool(name="bin", bufs=1) as p_bin,
            tc.tile_pool(name="ko", bufs=2) as p_ko,
            tc.tile_pool(name="vpl", bufs=1) as p_v,
            tc.tile_pool(name="cs", bufs=2) as p_cs,
            tc.tile_pool(name="ct", bufs=2) as p_ct,
            tc.tile_pool(name="rp", bufs=1) as p_rp,
            tc.tile_pool(name="it", bufs=1) as p_it,
            tc.tile_pool(name="pr", bufs=3) as p_pr,
            tc.tile_pool(name="o16", bufs=1) as p_o16,
            tc.tile_pool(name="cps", bufs=2, space="PSUM") as p_cps,
            tc.tile_pool(name="mps", bufs=2, space="PSUM") as p_mps,
            tc.tile_pool(name="ops", bufs=2, space="PSUM") as p_ops,
        ):
            wt = p_w.tile([128, (5 + BPC) * 128], F16)
            nc.sync.dma_start(wt[:], w_d[:])
            av = p_w.tile([128, 3], F32)
            nc.sync.dma_start(av[:], av_d[:])
            s2s_ap = av[:, 0:1]      # relu-affine bias s2 + 16
            piv_ap = av[:, 1:2]      # p//8 for select compare
            m16_ap = av[:, 2:3]      # -16.0 bias for the K-16 relu copy

            for img in range(BPC):
                bins = slot_bins[img]
                absent = [j for j in range(NQ) if j not in bins]

                xs = p_xs.tile([128, NV], F32, tag="xs")
                nc.sync.dma_start(xs[:], x_d[img])
                x16 = p_bin.tile([128, NV], F16, tag="x16")
                nc.sync.dma_start(x16[:], x16_d[img])
                xo = p_xo.tile([128, NO], F32, tag="xo")
                nc.sync.dma_start(xo[:], xo_d[img])

                k16 = p_bin.tile([128, NV], F16, tag="k16")
                _binchain(nc, p_bin, xs, k16, s2s_ap, m16_ap, s1, NV, "v")
                ko = p_ko.tile([128, NO], F16, tag="ko")
                _binchain(nc, p_xo, xo, ko, s2s_ap, m16_ap, s1, NO, "o")

                # scatter present bins: vj = (K == j) * x
                vts = {}
                for j in bins:
                    vj = p_v.tile([128, NV], F16, tag=f"vj{j}")
                    nc.vector.scalar_tensor_tensor(
                        vj[:], k16[:], float(j), x16[:],
                        ALU.is_equal, ALU.mult,
                    )
                    vts[j] = vj

                for ch in range(4):
                    cs = p_cs.tile([128, NQ * 512], F16, tag="cs")
                    cv = cs[:]
                    for j in absent:
                        zap = AP(cv.tensor, cv.offset + j * 8,
                                 [cv.ap[0], [128, 64], [1, 8]])
                        nc.gpsimd.memset(zap, 0.0)
                    for j in bins:
                        ps = p_cps.tile([128, 512], F32, tag="ps")
                        vt = vts[j][:]
                        for v in range(4):
                            da, db = v >> 1, v & 1
                            rhs = AP(
                                vt.tensor,
                                vt.offset + (ch * 16 + da) * BV + db,
                                [[NV, 128], [BV, 16], [1, BO]],
                            )
                            nc.tensor.matmul(
                                ps[:], wt[:, v * 128 : (v + 1) * 128], rhs,
                                start=(v == 0), stop=(v == 3),
                            )
                        dst = AP(cv.tensor, cv.offset + j * 8,
                                 [cv.ap[0], [128, 64], [1, 8]])
                        nc.scalar.copy(dst, ps[:])

                    pools = (p_ct, p_rp, p_it, p_pr, p_o16, p_mps, p_ops)
                    for h in range(2):
                        _strip_half(nc, img, ch * 64 + h * 32, wt, ko, cs, h,
                                    piv_ap, pools, o_d)

    nc.compile()
    return nc


# ------------------------------------------------------- host packing
def pack_inputs(x):
    imgs = x[:, 0]
    xpad = np.pad(imgs, ((0, 0), (2, 6), (2, 14)))      # [64, 520, 528]
    xb = (
        xpad.reshape(B, AV, 8, BV, 16)
        .transpose(0, 2, 4, 1, 3)
        .reshape(B, 128, NV)
    )
    return np.ascontiguousarray(xb)


def pack_outgrid(x):
    imgs = x[:, 0]                                       # [64, 512, 512]
    xo = (
        imgs.reshape(B, 64, 8, 32, 16)
        .transpose(0, 2, 4, 1, 3)
        .reshape(B, 128, NO)
    )
    return np.ascontiguousarray(xo)


def unpack_outputs(res_list, groups):
    out = np.empty((B, 1, HH, WW), np.float32)
    for c in range(NCORES):
        ob = res_list[c]["out"].astype(np.float32)       # [BPC, 8, NG*128]
        o = ob.reshape(BPC, 8, 64, 4, 8, 16)
        o = o.transpose(0, 2, 4, 3, 1, 5)
        o = o.reshape(BPC, HH, WW)
        for s in range(BPC):
            out[groups[s][c], 0] = o[s]
    return out


def make_av(s2):
    av = np.zeros((128, 3), np.float32)
    av[:, 0] = s2
    av[:, 1] = (np.arange(128) // 8).astype(np.float32)
    av[:, 2] = -16.0
    return np.ascontiguousarray(av)


def prepare(x, co, filt):
    import jax.numpy as jnp

    x = np.asarray(x, np.float32)
    co = np.asarray(co, np.float32)
    filt = np.asarray(filt, np.float32)

    m = np.float32(np.asarray(jnp.exp(np.float32(x.min()))))
    M = np.float32(np.asarray(jnp.exp(np.float32(x.max()))))
    s1 = np.float32(np.float64(16.0) / np.float64(M))
    s2 = np.float32(-(np.float64(16.0) * np.float64(m) / np.float64(M) + EPS))

    groups, slot_bins = group_images(x, m, M)
    xb = pack_inputs(x)
    xb16 = xb.astype(np.float16)
    xog = pack_outgrid(x)
    wts = build_weights(co, filt, slot_bins)
    av = make_av(s2)

    in_maps = []
    for c in range(NCORES):
        sel = np.stack([xb[groups[s][c]] for s in range(BPC)])
        sel16 = np.stack([xb16[groups[s][c]] for s in range(BPC)])
        selo = np.stack([xog[groups[s][c]] for s in range(BPC)])
        in_maps.append({"x": np.ascontiguousarray(sel),
                        "x16": np.ascontiguousarray(sel16),
                        "xo": np.ascontiguousarray(selo),
                        "wt": wts, "av": av})
    return s1, s2, slot_bins, groups, in_maps


def kernel(x, co_matrix, spatial_filter):
    s1, s2, slot_bins, groups, in_maps = prepare(x, co_matrix, spatial_filter)
    nc = build_program(s1, s2, slot_bins)
    res = bass_utils.run_bass_kernel_spmd(nc, in_maps, core_ids=list(range(NCORES)))
    return unpack_outputs(res.results, groups)
